# revision 5
# baseline (speedup 1.0000x reference)
"""Trainium2 Bass kernel v3 for nn_CodeLinearAttention (B=2, T=2048, D=1024,
H=16, HD=64, C=16) on 8 NeuronCores.

Sharding: core c -> batch b = c//4, head group g = c%4 (4 heads per core).
All matmul operands bf16 (PSUM accumulates fp32); DMA payloads bf16.

Structure (per core; heads j=0..3 sit in 32-wide partition/col blocks, the
16 pad lanes carry exp(0)=1 from zero code columns so everything is finite):
  P1   qkv projection -> qkT[p] [128,T] (hd-major pairs), v_sb [t,(h d)].
  P2a  k-code projection in padded [c, t] layout -> ekT [128,T] = exp(.).
  P2g  global cross-chunk cumsum carries: per-chunk column totals via one
       segmented reduce of ekT, inclusive scan over chunks, PE transpose
       -> car_sb [16, 128]; consumed via one-hot K=16 matmuls (no serial
       carry chain).
  A    per chunk: q-code proj + exp (t-major), ekT chunk transpose ->
       ekTC[:,chunk], kcum = triu-matmul + carry matmul, r = 1/kcum,
       S_q + 1/S_q, qn = eq*(1/S_q)*r, transpose -> qnT[:,chunk].
       Chunks fully independent -> deep pipelining.
  B    per chunk: AT = ek^T qn (one row-tiled matmul + masked mul per
       head, each owning a PSUM bank), xo per-head tiles = V^T AT +
       KV^T qn, KV state in SBUF fp32 (col-tiled delta matmuls), fused
       output projection (P4) + DMA per chunk.
The loop body is unrolled 2x with double-buffered persistent tensors so
iteration n+1's projection phases overlap iteration n's attention phases.
The post-softmax *scale is folded into w_outT on the host (exact pow2).
"""

import sys

sys.path.insert(0, "/opt/trn_rl_repo")

from contextlib import ExitStack

import numpy as np
import ml_dtypes

import concourse.bacc as bacc
import concourse.tile as tile
from concourse import mybir

F32 = mybir.dt.float32
BF16 = mybir.dt.bfloat16
AX = mybir.AxisListType
OP = mybir.AluOpType
AF = mybir.ActivationFunctionType

B, T, D, NHEAD, HD, C = 2, 2048, 1024, 16, 64, 16
HC = 4  # heads per core
CH = 128  # attention chunk
NCH = T // CH  # 16
TC = 512  # big t chunk for projections
NTC = T // TC  # 4
SCALE = HD ** -0.5  # 0.125
N_CORES = 8


def make_pools(tc_, ctx):
    pools = {}
    pools["const"] = ctx.enter_context(tc_.tile_pool(name="const", bufs=1))
    # persistent per-iteration tensors are double-buffered so the unrolled
    # pair of iterations can overlap
    pools["persist"] = ctx.enter_context(tc_.tile_pool(name="persist", bufs=2))
    pools["xin"] = ctx.enter_context(tc_.tile_pool(name="xin", bufs=10))
    pools["at"] = ctx.enter_context(tc_.tile_pool(name="at", bufs=3))
    pools["td"] = ctx.enter_context(tc_.tile_pool(name="td", bufs=4))
    pools["sq"] = ctx.enter_context(tc_.tile_pool(name="sq", bufs=4))
    pools["r"] = ctx.enter_context(tc_.tile_pool(name="r", bufs=4))
    pools["qn"] = ctx.enter_context(tc_.tile_pool(name="qn", bufs=4))
    pools["kvb"] = ctx.enter_context(tc_.tile_pool(name="kvb", bufs=2))
    pools["outcp"] = ctx.enter_context(tc_.tile_pool(name="outcp", bufs=2))

    # Single 8-slot PSUM pool (one tag): every tile occupies a full bank and
    # allocations round-robin through all 8 banks. Row-tiled matmuls thereby
    # always own a bank exclusively (Tile's WAR/RAW deps serialize any bank
    # reuse), which hardware requires — concurrent row tiles on one PSUM bank
    # crash the exec unit.
    pools["ps"] = ctx.enter_context(tc_.tile_pool(name="ps", bufs=8, space="PSUM"))
    return pools


def load_consts(nc, pools, io):
    xT, wqkvT, codeT4, woutT, maskT, iden, onehot, out = io
    const = pools["const"]
    wq_sb = []
    for dci in range(8):
        w = const.tile([128, 768], BF16, tag=f"wq{dci}", name=f"wq{dci}")
        nc.sync.dma_start(w[:], wqkvT[dci * 128 : (dci + 1) * 128, :])
        wq_sb.append(w)
    code_sb = const.tile([128, 128], BF16)
    nc.sync.dma_start(code_sb[:], codeT4)
    wout_sb = []
    for kk in range(2):
        w = const.tile([128, 1024], BF16, tag=f"wout{kk}", name=f"wout{kk}")
        nc.sync.dma_start(w[:], woutT[kk * 128 : (kk + 1) * 128, :])
        wout_sb.append(w)
    mask_sb = const.tile([128, 512], BF16)  # triu tiled 4x along free
    nc.sync.dma_start(mask_sb[:], maskT)
    iden_sb = const.tile([128, 128], BF16)
    nc.sync.dma_start(iden_sb[:], iden)
    oh_sb = const.tile([16, NCH * 128], BF16)
    nc.sync.dma_start(oh_sb[:], onehot)
    return wq_sb, code_sb, wout_sb, mask_sb, iden_sb, oh_sb


def emit_iter(nc, pools, consts, io, phases=9):
    xT, wqkvT, codeT4, woutT, maskT, iden, onehot, out = io
    wq_sb, code_sb, wout_sb, mask_sb, iden_sb, oh_sb = consts
    persist = pools["persist"]
    xin = pools["xin"]
    at_pool = pools["at"]
    td_pool = pools["td"]
    sq_pool = pools["sq"]
    r_pool = pools["r"]
    qn_pool = pools["qn"]
    kvb_pool = pools["kvb"]
    outcp_pool = pools["outcp"]
    ps = pools["ps"]

    def pstile(shape, dtype, name):
        return ps.tile(shape, dtype, tag="ps", name=name)

    # ---- persistent SBUF tensors (rotate between the two unrolled bodies) ----
    qkT = [persist.tile([128, T], BF16, tag=f"qkT{j}", name=f"qkT{j}") for j in range(4)]
    v_sb = persist.tile([128, NCH * 256], BF16, tag="v_sb", name="v_sb")
    ekT = persist.tile([128, T], BF16, tag="ekT")
    ekTC = persist.tile([128, T], BF16, tag="ekTC")
    qnT = persist.tile([128, T], BF16, tag="qnT")
    xoT = [persist.tile([128, T], BF16, tag=f"xoT{k}", name=f"xoT{k}") for k in range(2)]
    tots_sb = persist.tile([128, NCH], F32, tag="tots_sb")
    car_cT = persist.tile([128, NCH], BF16, tag="car_cT")
    car_sb = persist.tile([16, 128], BF16, tag="car_sb")

    # ================= P1: qkv projection =================
    for tci in range(NTC):
        xts = []
        for dci in range(8):
            xt = xin.tile([128, TC], BF16, tag="xt", name="xt")
            nc.sync.dma_start(
                xt[:], xT[dci * 128 : (dci + 1) * 128, tci * TC : (tci + 1) * TC]
            )
            xts.append(xt)
        for j in range(4):  # q pair0, q pair1, k pair0, k pair1
            pqk = pstile([128, TC], F32, "psqk")
            for dci in range(8):
                nc.tensor.matmul(
                    pqk[:],
                    lhsT=wq_sb[dci][:, j * 128 : (j + 1) * 128],
                    rhs=xts[dci][:],
                    start=(dci == 0),
                    stop=(dci == 7),
                )
            nc.scalar.copy(out=qkT[j][:, tci * TC : (tci + 1) * TC], in_=pqk[:])
        for sub in range(4):
            pv = pstile([128, 256], F32, "psv")
            for dci in range(8):
                nc.tensor.matmul(
                    pv[:],
                    lhsT=xts[dci][:, sub * 128 : (sub + 1) * 128],
                    rhs=wq_sb[dci][:, 512:768],
                    start=(dci == 0),
                    stop=(dci == 7),
                )
            ci = tci * 4 + sub
            nc.vector.tensor_copy(out=v_sb[:, ci * 256 : (ci + 1) * 256], in_=pv[:])

    if phases < 2:
        nc.sync.dma_start(out[0:128, :], qkT[0][:, 0:1024])
        return

    # ====== P2a: k-code projection in padded [c, t] layout, exp ======
    for tci in range(NTC):
        ts = slice(tci * TC, (tci + 1) * TC)
        pck = pstile([128, TC], F32, "pck")
        for j in range(HC):
            nc.tensor.matmul(
                pck[32 * j : 32 * j + 32, :],
                lhsT=code_sb[:, 32 * j : 32 * j + 32],
                rhs=qkT[2 + j // 2][:, ts],
                start=True,
                stop=True,
                tile_position=(0, 32 * j),
            )
        nc.scalar.activation(ekT[:, ts], pck[:], AF.Exp, scale=SCALE)
        # per-chunk column totals for this span (segmented free-axis reduce)
        nc.vector.tensor_reduce(
            tots_sb[:, tci * 4 : (tci + 1) * 4],
            ekT[:, ts].rearrange("p (i t) -> p i t", t=CH),
            axis=AX.X,
            op=OP.add,
        )

    if phases < 3:
        nc.sync.dma_start(out[0:128, :], ekT[:, 0:1024])
        return

    # ====== P2g: inclusive scan over chunks -> carries [16, 128] ======
    nc.vector.tensor_tensor_scan(
        out=car_cT[:],
        data0=tots_sb[:],
        data1=tots_sb[:],
        initial=0.0,
        op0=OP.add,
        op1=OP.bypass,
    )
    car_tp = pstile([16, 128], BF16, "car_tp")
    nc.tensor.transpose(car_tp[:], car_cT[:], iden_sb[:])
    nc.vector.tensor_copy(out=car_sb[:], in_=car_tp[:])

    # ============ Loop A: per-chunk normalization -> qnT, ekTC ============
    for i in range(NCH):
        tsl = slice(i * CH, (i + 1) * CH)

        pack = pstile([128, 256], F32, "pack")
        pcq, kcum = pack[:, 0:128], pack[:, 128:256]
        for j in range(HC):
            nc.tensor.matmul(
                pcq[:, 32 * j : 32 * j + 32],
                lhsT=qkT[j // 2][:, tsl],
                rhs=code_sb[:, 32 * j : 32 * j + 32],
                start=True,
                stop=True,
            )
        eq_td = td_pool.tile([128, 128], BF16, tag="td", name="eq_td")
        nc.scalar.activation(eq_td[:], pcq[:], AF.Exp, scale=SCALE)

        trp = pstile([128, 256], BF16, "trp")
        tp, tq = trp[:, 0:128], trp[:, 128:256]
        nc.tensor.transpose(tp[:], ekT[:, tsl], iden_sb[:])
        nc.scalar.copy(out=ekTC[:, tsl], in_=tp[:])

        nc.tensor.matmul(
            kcum[:],
            lhsT=mask_sb[:, 0:128],
            rhs=ekTC[:, tsl],
            start=True,
            stop=(i == 0),
        )
        if i > 0:
            nc.tensor.matmul(
                kcum[:],
                lhsT=oh_sb[:, (i - 1) * 128 : i * 128],
                rhs=car_sb[:],
                start=False,
                stop=True,
            )
        r = r_pool.tile([128, 128], F32, tag="r", name="r")
        nc.vector.reciprocal(r[:], kcum[:])

        sq = sq_pool.tile([128, 8], F32, tag="sq", name="sq")
        nc.vector.tensor_reduce(
            sq[:, 0:4],
            eq_td[:].rearrange("p (h c) -> p h c", c=32)[:, :, 0:16],
            axis=AX.X,
            op=OP.add,
        )
        nc.vector.reciprocal(sq[:, 4:8], sq[:, 0:4])

        qn_tc = qn_pool.tile([128, 128], BF16, tag="qn", name="qn_tc")
        for j in range(HC):
            csl = slice(32 * j, 32 * j + 32)
            nc.vector.scalar_tensor_tensor(
                out=qn_tc[:, csl],
                in0=eq_td[:, csl],
                scalar=sq[:, 4 + j : 5 + j],
                in1=r[:, csl],
                op0=OP.mult,
                op1=OP.mult,
            )
        nc.tensor.transpose(tq[:], qn_tc[:], iden_sb[:])
        nc.scalar.copy(out=qnT[:, tsl], in_=tq[:])

    if phases < 4:
        nc.sync.dma_start(out[0:128, :], qnT[:, 0:1024])
        return

    # ============ Loop B: attention + fused output projection ============
    kv_sb = persist.tile([128, 64], F32, tag="kv_sb")
    for i in range(NCH):
        tsl = slice(i * CH, (i + 1) * CH)

        # KV snapshot in bf16 (state after chunks < i)
        kv_bf = None
        if i > 0:
            kv_bf = kvb_pool.tile([128, 64], BF16, tag="kvb", name="kv_bf")
            nc.gpsimd.tensor_copy(out=kv_bf[:], in_=kv_sb[:])

        # AT per head; each row-tiled matmul owns its own PSUM bank
        at_tiles = [pstile([128, 128], F32, f"atp{j}") for j in range(HC)]
        for j in range(HC):
            nc.tensor.matmul(
                at_tiles[j][:],
                lhsT=ekT[32 * j : 32 * j + 16, tsl],
                rhs=qnT[32 * j : 32 * j + 16, tsl],
                start=True,
                stop=True,
                tile_position=(32 * j, 0),
            )
        at_sb = at_pool.tile([128, 512], BF16, tag="at", name="at_sb")
        for j in range(HC):
            nc.vector.tensor_mul(
                at_sb[:, 128 * j : 128 * (j + 1)], at_tiles[j][:], mask_sb[:, 0:128]
            )
        if phases < 5:
            continue

        # xo per head: V^T AT (+ KV^T qn), one PSUM bank per head
        xo_tiles = [pstile([64, 128], F32, f"xop{j}") for j in range(HC)]
        for j in range(HC):
            nc.tensor.matmul(
                xo_tiles[j][:],
                lhsT=v_sb[:, i * 256 + 64 * j : i * 256 + 64 * (j + 1)],
                rhs=at_sb[:, 128 * j : 128 * (j + 1)],
                start=True,
                stop=(i == 0 or phases < 6),
            )
            if i > 0 and phases >= 6:
                nc.tensor.matmul(
                    xo_tiles[j][:],
                    lhsT=kv_bf[32 * j : 32 * j + 16, :],
                    rhs=qnT[32 * j : 32 * j + 16, tsl],
                    start=False,
                    stop=True,
                    tile_position=(32 * j, 0),
                )
        for j in range(HC):
            half, po = j // 2, 64 * (j % 2)
            nc.scalar.copy(
                out=xoT[half][po : po + 64, tsl], in_=xo_tiles[j][:]
            )

        # KV state += Ek_i^T V_i (col-tiled matmuls; SBUF fp32 accumulator)
        if phases >= 6 and i < NCH - 1:
            kvp = pstile([128, 64], F32, "kvp")
            for j in range(HC):
                nc.tensor.matmul(
                    kvp[32 * j : 32 * j + 32, :],
                    lhsT=ekTC[:, i * CH + 32 * j : i * CH + 32 * j + 32],
                    rhs=v_sb[:, i * 256 + 64 * j : i * 256 + 64 * (j + 1)],
                    start=True,
                    stop=True,
                    tile_position=(0, 32 * j),
                )
            if i == 0:
                nc.vector.tensor_copy(out=kv_sb[:], in_=kvp[:])
            else:
                nc.vector.tensor_add(kv_sb[:], kv_sb[:], kvp[:])

        if phases < 9:
            if i == NCH - 1:
                nc.sync.dma_start(out[0:128, :], xoT[0][:, 0:1024])
            continue
        # fused output projection for this chunk
        ocp = outcp_pool.tile([128, 1024], BF16, tag="ocp", name="ocp")
        for nh in range(2):
            op = pstile([128, 512], F32, "op")
            nc.tensor.matmul(
                op[:],
                lhsT=xoT[0][:, tsl],
                rhs=wout_sb[0][:, nh * 512 : (nh + 1) * 512],
                start=True,
                stop=False,
            )
            nc.tensor.matmul(
                op[:],
                lhsT=xoT[1][:, tsl],
                rhs=wout_sb[1][:, nh * 512 : (nh + 1) * 512],
                start=False,
                stop=True,
            )
            if nh == 0:
                nc.scalar.copy(out=ocp[:, nh * 512 : (nh + 1) * 512], in_=op[:])
            else:
                nc.vector.tensor_copy(out=ocp[:, nh * 512 : (nh + 1) * 512], in_=op[:])
        nc.sync.dma_start(out[tsl, :], ocp[:])


def build(n_iter: int = 1, phases: int = 9):
    nc = bacc.Bacc("TRN2", target_bir_lowering=False, debug=False, num_devices=N_CORES)
    xT = nc.dram_tensor("xT", [D, T], BF16, kind="ExternalInput").ap()
    wqkvT = nc.dram_tensor("wqkvT", [D, 768], BF16, kind="ExternalInput").ap()
    codeT4 = nc.dram_tensor("codeT4", [128, 128], BF16, kind="ExternalInput").ap()
    woutT = nc.dram_tensor("woutT", [256, 1024], BF16, kind="ExternalInput").ap()
    maskT = nc.dram_tensor("maskT", [128, 512], BF16, kind="ExternalInput").ap()
    iden = nc.dram_tensor("iden", [128, 128], BF16, kind="ExternalInput").ap()
    onehot = nc.dram_tensor("onehot", [16, NCH * 128], BF16, kind="ExternalInput").ap()
    out = nc.dram_tensor("partial", [T, D], BF16, kind="ExternalOutput").ap()
    io = (xT, wqkvT, codeT4, woutT, maskT, iden, onehot, out)

    with tile.TileContext(nc) as tc_, ExitStack() as ctx:
        pools = make_pools(tc_, ctx)
        consts = load_consts(nc, pools, io)
        if n_iter <= 4:
            for _ in range(n_iter):
                emit_iter(nc, pools, consts, io, phases)
        else:
            assert n_iter % 2 == 0, "n_iter must be even (2x-unrolled loop)"
            with tc_.For_i(0, n_iter // 2, 1):
                emit_iter(nc, pools, consts, io, phases)
                emit_iter(nc, pools, consts, io, phases)
    nc.compile()
    return nc


def make_in_maps(x, w_qkv, w_out, fc_code):
    x = np.asarray(x, dtype=np.float32)
    w_qkv = np.asarray(w_qkv, dtype=np.float32)
    w_out = np.asarray(w_out, dtype=np.float32)
    fc_code = np.asarray(fc_code, dtype=np.float32)
    bf = ml_dtypes.bfloat16

    mask = np.tile(np.triu(np.ones((128, 128), dtype=np.float32)), (1, 4))
    iden = np.eye(128, dtype=np.float32)
    onehot = np.zeros((16, NCH * 128), dtype=np.float32)
    for i in range(NCH):
        onehot[i, i * 128 : (i + 1) * 128] = 1.0
    xTs = [np.ascontiguousarray(x[b].T).astype(bf) for b in range(B)]

    in_maps = []
    for core in range(N_CORES):
        b, g = core // HC, core % HC
        hs = [g * HC + j for j in range(HC)]
        rows = (
            [w_qkv[h * HD : (h + 1) * HD] for h in hs]
            + [w_qkv[D + h * HD : D + (h + 1) * HD] for h in hs]
            + [w_qkv[2 * D + h * HD : 2 * D + (h + 1) * HD] for h in hs]
        )
        wqkvT = np.ascontiguousarray(np.concatenate(rows, axis=0).T)  # (1024, 768)
        codeT4 = np.zeros((128, 128), dtype=np.float32)
        for j, h in enumerate(hs):
            hh = j % 2  # position within the hd pair
            ct = fc_code[0, h].T  # (64, 16)
            codeT4[64 * hh : 64 * hh + 64, 32 * j : 32 * j + 16] = ct
        woutT = np.ascontiguousarray(
            np.concatenate([w_out[:, h * HD : (h + 1) * HD].T for h in hs], axis=0)
        ) * np.float32(SCALE)  # (256, 1024), post-softmax scale folded in
        in_maps.append(
            {
                "xT": xTs[b],
                "wqkvT": wqkvT.astype(bf),
                "codeT4": codeT4.astype(bf),
                "woutT": woutT.astype(bf),
                "maskT": mask.astype(bf),
                "iden": iden.astype(bf),
                "onehot": onehot.astype(bf),
            }
        )
    return in_maps


def gather(results):
    out = np.zeros((B, T, D), dtype=np.float32)
    for core in range(N_CORES):
        out[core // HC] += np.asarray(results[core]["partial"], dtype=np.float32)
    return out


_NC_CACHE = {}


def kernel(x, w_qkv, w_out, fc_code):
    from concourse.bass_utils import run_bass_kernel_spmd

    if 1 not in _NC_CACHE:
        _NC_CACHE[1] = build(1)
    nc = _NC_CACHE[1]
    in_maps = make_in_maps(x, w_qkv, w_out, fc_code)
    res = run_bass_kernel_spmd(nc, in_maps, list(range(N_CORES)))
    return gather(res.results)


# revision 6
# speedup vs baseline: 4.9112x; 4.9112x over previous
"""Trainium2 Bass kernel v3 for nn_CodeLinearAttention (B=2, T=2048, D=1024,
H=16, HD=64, C=16) on 8 NeuronCores.

Sharding: core c -> batch b = c//4, head group g = c%4 (4 heads per core).
All matmul operands bf16 (PSUM accumulates fp32); DMA payloads bf16.

Structure (per core; heads j=0..3 sit in 32-wide partition/col blocks, the
16 pad lanes carry exp(0)=1 from zero code columns so everything is finite):
  P1   qkv projection -> qkT[p] [128,T] (hd-major pairs), v_sb [t,(h d)].
  P2a  k-code projection in padded [c, t] layout -> ekT [128,T] = exp(.).
  P2g  global cross-chunk cumsum carries: per-chunk column totals via one
       segmented reduce of ekT, inclusive scan over chunks, PE transpose
       -> car_sb [16, 128]; consumed via one-hot K=16 matmuls (no serial
       carry chain).
  A    per chunk: q-code proj + exp (t-major), ekT chunk transpose ->
       ekTC[:,chunk], kcum = triu-matmul + carry matmul, r = 1/kcum,
       S_q + 1/S_q, qn = eq*(1/S_q)*r, transpose -> qnT[:,chunk].
       Chunks fully independent -> deep pipelining.
  B    per chunk: AT = ek^T qn (one row-tiled matmul + masked mul per
       head, each owning a PSUM bank), xo per-head tiles = V^T AT +
       KV^T qn, KV state in SBUF fp32 (col-tiled delta matmuls), fused
       output projection (P4) + DMA per chunk.
The loop body is unrolled 2x with double-buffered persistent tensors so
iteration n+1's projection phases overlap iteration n's attention phases.
The post-softmax *scale is folded into w_outT on the host (exact pow2).
"""

import os
import sys

sys.path.insert(0, "/opt/trn_rl_repo")
os.environ.setdefault("JAX_PLATFORMS", "axon")

from contextlib import ExitStack

import numpy as np
import ml_dtypes

import concourse.bacc as bacc
import concourse.tile as tile
from concourse import mybir

F32 = mybir.dt.float32
BF16 = mybir.dt.bfloat16
AX = mybir.AxisListType
OP = mybir.AluOpType
AF = mybir.ActivationFunctionType

B, T, D, NHEAD, HD, C = 2, 2048, 1024, 16, 64, 16
HC = 4  # heads per core
CH = 128  # attention chunk
NCH = T // CH  # 16
TC = 512  # big t chunk for projections
NTC = T // TC  # 4
SCALE = HD ** -0.5  # 0.125
N_CORES = 8


def make_pools(tc_, ctx):
    pools = {}
    pools["const"] = ctx.enter_context(tc_.tile_pool(name="const", bufs=1))
    # persistent per-iteration tensors are double-buffered so the unrolled
    # pair of iterations can overlap
    pools["persist"] = ctx.enter_context(tc_.tile_pool(name="persist", bufs=2))
    pools["xin"] = ctx.enter_context(tc_.tile_pool(name="xin", bufs=10))
    pools["at"] = ctx.enter_context(tc_.tile_pool(name="at", bufs=3))
    pools["td"] = ctx.enter_context(tc_.tile_pool(name="td", bufs=4))
    pools["sq"] = ctx.enter_context(tc_.tile_pool(name="sq", bufs=4))
    pools["r"] = ctx.enter_context(tc_.tile_pool(name="r", bufs=4))
    pools["qn"] = ctx.enter_context(tc_.tile_pool(name="qn", bufs=4))
    pools["kvb"] = ctx.enter_context(tc_.tile_pool(name="kvb", bufs=2))
    pools["outcp"] = ctx.enter_context(tc_.tile_pool(name="outcp", bufs=2))

    # Single 8-slot PSUM pool (one tag): every tile occupies a full bank and
    # allocations round-robin through all 8 banks. Row-tiled matmuls thereby
    # always own a bank exclusively (Tile's WAR/RAW deps serialize any bank
    # reuse), which hardware requires — concurrent row tiles on one PSUM bank
    # crash the exec unit.
    pools["ps"] = ctx.enter_context(tc_.tile_pool(name="ps", bufs=8, space="PSUM"))
    return pools


def load_consts(nc, pools, io):
    xT, wqkvT, codeT4, woutT, maskT, iden, onehot, out = io
    const = pools["const"]
    wq_sb = []
    for dci in range(8):
        w = const.tile([128, 768], BF16, tag=f"wq{dci}", name=f"wq{dci}")
        nc.sync.dma_start(w[:], wqkvT[dci * 128 : (dci + 1) * 128, :])
        wq_sb.append(w)
    code_sb = const.tile([128, 128], BF16)
    nc.sync.dma_start(code_sb[:], codeT4)
    wout_sb = []
    for kk in range(2):
        w = const.tile([128, 1024], BF16, tag=f"wout{kk}", name=f"wout{kk}")
        nc.sync.dma_start(w[:], woutT[kk * 128 : (kk + 1) * 128, :])
        wout_sb.append(w)
    mask_sb = const.tile([128, 512], BF16)  # triu tiled 4x along free
    nc.sync.dma_start(mask_sb[:], maskT)
    iden_sb = const.tile([128, 128], BF16)
    nc.sync.dma_start(iden_sb[:], iden)
    oh_sb = const.tile([16, NCH * 128], BF16)
    nc.sync.dma_start(oh_sb[:], onehot)
    return wq_sb, code_sb, wout_sb, mask_sb, iden_sb, oh_sb


def emit_iter(nc, pools, consts, io, phases=9):
    xT, wqkvT, codeT4, woutT, maskT, iden, onehot, out = io
    wq_sb, code_sb, wout_sb, mask_sb, iden_sb, oh_sb = consts
    persist = pools["persist"]
    xin = pools["xin"]
    at_pool = pools["at"]
    td_pool = pools["td"]
    sq_pool = pools["sq"]
    r_pool = pools["r"]
    qn_pool = pools["qn"]
    kvb_pool = pools["kvb"]
    outcp_pool = pools["outcp"]
    ps = pools["ps"]

    def pstile(shape, dtype, name):
        return ps.tile(shape, dtype, tag="ps", name=name)

    # ---- persistent SBUF tensors (rotate between the two unrolled bodies) ----
    qkT = [persist.tile([128, T], BF16, tag=f"qkT{j}", name=f"qkT{j}") for j in range(4)]
    v_sb = persist.tile([128, NCH * 256], BF16, tag="v_sb", name="v_sb")
    ekT = persist.tile([128, T], BF16, tag="ekT")
    ekTC = persist.tile([128, T], BF16, tag="ekTC")
    qnT = persist.tile([128, T], BF16, tag="qnT")
    xoT = [persist.tile([128, T], BF16, tag=f"xoT{k}", name=f"xoT{k}") for k in range(2)]
    tots_sb = persist.tile([128, NCH], F32, tag="tots_sb")
    car_cT = persist.tile([128, NCH], BF16, tag="car_cT")
    car_sb = persist.tile([16, 128], BF16, tag="car_sb")

    # ================= P1: qkv projection =================
    for tci in range(NTC):
        xts = []
        for dci in range(8):
            xt = xin.tile([128, TC], BF16, tag="xt", name="xt")
            nc.sync.dma_start(
                xt[:], xT[dci * 128 : (dci + 1) * 128, tci * TC : (tci + 1) * TC]
            )
            xts.append(xt)
        for j in range(4):  # q pair0, q pair1, k pair0, k pair1
            pqk = pstile([128, TC], F32, "psqk")
            for dci in range(8):
                nc.tensor.matmul(
                    pqk[:],
                    lhsT=wq_sb[dci][:, j * 128 : (j + 1) * 128],
                    rhs=xts[dci][:],
                    start=(dci == 0),
                    stop=(dci == 7),
                )
            nc.scalar.copy(out=qkT[j][:, tci * TC : (tci + 1) * TC], in_=pqk[:])
        for sub in range(4):
            pv = pstile([128, 256], F32, "psv")
            for dci in range(8):
                nc.tensor.matmul(
                    pv[:],
                    lhsT=xts[dci][:, sub * 128 : (sub + 1) * 128],
                    rhs=wq_sb[dci][:, 512:768],
                    start=(dci == 0),
                    stop=(dci == 7),
                )
            ci = tci * 4 + sub
            nc.vector.tensor_copy(out=v_sb[:, ci * 256 : (ci + 1) * 256], in_=pv[:])

    if phases < 2:
        nc.sync.dma_start(out[0:128, :], qkT[0][:, 0:1024])
        return

    # ====== P2a: k-code projection in padded [c, t] layout, exp ======
    for tci in range(NTC):
        ts = slice(tci * TC, (tci + 1) * TC)
        pck = pstile([128, TC], F32, "pck")
        for j in range(HC):
            nc.tensor.matmul(
                pck[32 * j : 32 * j + 32, :],
                lhsT=code_sb[:, 32 * j : 32 * j + 32],
                rhs=qkT[2 + j // 2][:, ts],
                start=True,
                stop=True,
                tile_position=(0, 32 * j),
            )
        nc.scalar.activation(ekT[:, ts], pck[:], AF.Exp, scale=SCALE)
        # per-chunk column totals for this span (segmented free-axis reduce)
        nc.vector.tensor_reduce(
            tots_sb[:, tci * 4 : (tci + 1) * 4],
            ekT[:, ts].rearrange("p (i t) -> p i t", t=CH),
            axis=AX.X,
            op=OP.add,
        )

    if phases < 3:
        nc.sync.dma_start(out[0:128, :], ekT[:, 0:1024])
        return

    # ====== P2g: inclusive scan over chunks -> carries [16, 128] ======
    nc.vector.tensor_tensor_scan(
        out=car_cT[:],
        data0=tots_sb[:],
        data1=tots_sb[:],
        initial=0.0,
        op0=OP.add,
        op1=OP.bypass,
    )
    car_tp = pstile([16, 128], BF16, "car_tp")
    nc.tensor.transpose(car_tp[:], car_cT[:], iden_sb[:])
    nc.vector.tensor_copy(out=car_sb[:], in_=car_tp[:])

    # ============ Loop A: per-chunk normalization -> qnT, ekTC ============
    for i in range(NCH):
        tsl = slice(i * CH, (i + 1) * CH)

        pack = pstile([128, 256], F32, "pack")
        pcq, kcum = pack[:, 0:128], pack[:, 128:256]
        for j in range(HC):
            nc.tensor.matmul(
                pcq[:, 32 * j : 32 * j + 32],
                lhsT=qkT[j // 2][:, tsl],
                rhs=code_sb[:, 32 * j : 32 * j + 32],
                start=True,
                stop=True,
            )
        eq_td = td_pool.tile([128, 128], BF16, tag="td", name="eq_td")
        nc.scalar.activation(eq_td[:], pcq[:], AF.Exp, scale=SCALE)

        trp = pstile([128, 256], BF16, "trp")
        tp, tq = trp[:, 0:128], trp[:, 128:256]
        nc.tensor.transpose(tp[:], ekT[:, tsl], iden_sb[:])
        nc.scalar.copy(out=ekTC[:, tsl], in_=tp[:])

        nc.tensor.matmul(
            kcum[:],
            lhsT=mask_sb[:, 0:128],
            rhs=ekTC[:, tsl],
            start=True,
            stop=(i == 0),
        )
        if i > 0:
            nc.tensor.matmul(
                kcum[:],
                lhsT=oh_sb[:, (i - 1) * 128 : i * 128],
                rhs=car_sb[:],
                start=False,
                stop=True,
            )
        r = r_pool.tile([128, 128], F32, tag="r", name="r")
        nc.vector.reciprocal(r[:], kcum[:])

        sq = sq_pool.tile([128, 8], F32, tag="sq", name="sq")
        nc.vector.tensor_reduce(
            sq[:, 0:4],
            eq_td[:].rearrange("p (h c) -> p h c", c=32)[:, :, 0:16],
            axis=AX.X,
            op=OP.add,
        )
        nc.vector.reciprocal(sq[:, 4:8], sq[:, 0:4])

        qn_tc = qn_pool.tile([128, 128], BF16, tag="qn", name="qn_tc")
        for j in range(HC):
            csl = slice(32 * j, 32 * j + 32)
            nc.vector.scalar_tensor_tensor(
                out=qn_tc[:, csl],
                in0=eq_td[:, csl],
                scalar=sq[:, 4 + j : 5 + j],
                in1=r[:, csl],
                op0=OP.mult,
                op1=OP.mult,
            )
        nc.tensor.transpose(tq[:], qn_tc[:], iden_sb[:])
        nc.scalar.copy(out=qnT[:, tsl], in_=tq[:])

    if phases < 4:
        nc.sync.dma_start(out[0:128, :], qnT[:, 0:1024])
        return

    # ============ Loop B: attention + fused output projection ============
    kv_sb = persist.tile([128, 64], F32, tag="kv_sb")
    for i in range(NCH):
        tsl = slice(i * CH, (i + 1) * CH)

        # KV snapshot in bf16 (state after chunks < i)
        kv_bf = None
        if i > 0:
            kv_bf = kvb_pool.tile([128, 64], BF16, tag="kvb", name="kv_bf")
            nc.gpsimd.tensor_copy(out=kv_bf[:], in_=kv_sb[:])

        # AT per head; each row-tiled matmul owns its own PSUM bank
        at_tiles = [pstile([128, 128], F32, f"atp{j}") for j in range(HC)]
        for j in range(HC):
            nc.tensor.matmul(
                at_tiles[j][:],
                lhsT=ekT[32 * j : 32 * j + 16, tsl],
                rhs=qnT[32 * j : 32 * j + 16, tsl],
                start=True,
                stop=True,
                tile_position=(32 * j, 0),
            )
        at_sb = at_pool.tile([128, 512], BF16, tag="at", name="at_sb")
        for j in range(HC):
            nc.vector.tensor_mul(
                at_sb[:, 128 * j : 128 * (j + 1)], at_tiles[j][:], mask_sb[:, 0:128]
            )
        if phases < 5:
            continue

        # xo per head: V^T AT (+ KV^T qn), one PSUM bank per head
        xo_tiles = [pstile([64, 128], F32, f"xop{j}") for j in range(HC)]
        for j in range(HC):
            nc.tensor.matmul(
                xo_tiles[j][:],
                lhsT=v_sb[:, i * 256 + 64 * j : i * 256 + 64 * (j + 1)],
                rhs=at_sb[:, 128 * j : 128 * (j + 1)],
                start=True,
                stop=(i == 0 or phases < 6),
            )
            if i > 0 and phases >= 6:
                nc.tensor.matmul(
                    xo_tiles[j][:],
                    lhsT=kv_bf[32 * j : 32 * j + 16, :],
                    rhs=qnT[32 * j : 32 * j + 16, tsl],
                    start=False,
                    stop=True,
                    tile_position=(32 * j, 0),
                )
        for j in range(HC):
            half, po = j // 2, 64 * (j % 2)
            nc.scalar.copy(
                out=xoT[half][po : po + 64, tsl], in_=xo_tiles[j][:]
            )

        # KV state += Ek_i^T V_i (col-tiled matmuls; SBUF fp32 accumulator)
        if phases >= 6 and i < NCH - 1:
            kvp = pstile([128, 64], F32, "kvp")
            for j in range(HC):
                nc.tensor.matmul(
                    kvp[32 * j : 32 * j + 32, :],
                    lhsT=ekTC[:, i * CH + 32 * j : i * CH + 32 * j + 32],
                    rhs=v_sb[:, i * 256 + 64 * j : i * 256 + 64 * (j + 1)],
                    start=True,
                    stop=True,
                    tile_position=(0, 32 * j),
                )
            if i == 0:
                nc.vector.tensor_copy(out=kv_sb[:], in_=kvp[:])
            else:
                nc.vector.tensor_add(kv_sb[:], kv_sb[:], kvp[:])

        if phases < 9:
            if i == NCH - 1:
                nc.sync.dma_start(out[0:128, :], xoT[0][:, 0:1024])
            continue
        # fused output projection for this chunk
        ocp = outcp_pool.tile([128, 1024], BF16, tag="ocp", name="ocp")
        for nh in range(2):
            op = pstile([128, 512], F32, "op")
            nc.tensor.matmul(
                op[:],
                lhsT=xoT[0][:, tsl],
                rhs=wout_sb[0][:, nh * 512 : (nh + 1) * 512],
                start=True,
                stop=False,
            )
            nc.tensor.matmul(
                op[:],
                lhsT=xoT[1][:, tsl],
                rhs=wout_sb[1][:, nh * 512 : (nh + 1) * 512],
                start=False,
                stop=True,
            )
            if nh == 0:
                nc.scalar.copy(out=ocp[:, nh * 512 : (nh + 1) * 512], in_=op[:])
            else:
                nc.vector.tensor_copy(out=ocp[:, nh * 512 : (nh + 1) * 512], in_=op[:])
        nc.sync.dma_start(out[tsl, :], ocp[:])


def build(n_iter: int = 1, phases: int = 9):
    nc = bacc.Bacc("TRN2", target_bir_lowering=False, debug=False, num_devices=N_CORES)
    xT = nc.dram_tensor("xT", [D, T], BF16, kind="ExternalInput").ap()
    wqkvT = nc.dram_tensor("wqkvT", [D, 768], BF16, kind="ExternalInput").ap()
    codeT4 = nc.dram_tensor("codeT4", [128, 128], BF16, kind="ExternalInput").ap()
    woutT = nc.dram_tensor("woutT", [256, 1024], BF16, kind="ExternalInput").ap()
    maskT = nc.dram_tensor("maskT", [128, 512], BF16, kind="ExternalInput").ap()
    iden = nc.dram_tensor("iden", [128, 128], BF16, kind="ExternalInput").ap()
    onehot = nc.dram_tensor("onehot", [16, NCH * 128], BF16, kind="ExternalInput").ap()
    out = nc.dram_tensor("partial", [T, D], BF16, kind="ExternalOutput").ap()
    io = (xT, wqkvT, codeT4, woutT, maskT, iden, onehot, out)

    with tile.TileContext(nc) as tc_, ExitStack() as ctx:
        pools = make_pools(tc_, ctx)
        consts = load_consts(nc, pools, io)
        if n_iter <= 4:
            for _ in range(n_iter):
                emit_iter(nc, pools, consts, io, phases)
        else:
            assert n_iter % 2 == 0, "n_iter must be even (2x-unrolled loop)"
            with tc_.For_i(0, n_iter // 2, 1):
                emit_iter(nc, pools, consts, io, phases)
                emit_iter(nc, pools, consts, io, phases)
    nc.compile()
    return nc


def make_in_maps(x, w_qkv, w_out, fc_code):
    x = np.asarray(x, dtype=np.float32)
    w_qkv = np.asarray(w_qkv, dtype=np.float32)
    w_out = np.asarray(w_out, dtype=np.float32)
    fc_code = np.asarray(fc_code, dtype=np.float32)
    bf = ml_dtypes.bfloat16

    mask = np.tile(np.triu(np.ones((128, 128), dtype=np.float32)), (1, 4))
    iden = np.eye(128, dtype=np.float32)
    onehot = np.zeros((16, NCH * 128), dtype=np.float32)
    for i in range(NCH):
        onehot[i, i * 128 : (i + 1) * 128] = 1.0
    xTs = [np.ascontiguousarray(x[b].T).astype(bf) for b in range(B)]

    in_maps = []
    for core in range(N_CORES):
        b, g = core // HC, core % HC
        hs = [g * HC + j for j in range(HC)]
        rows = (
            [w_qkv[h * HD : (h + 1) * HD] for h in hs]
            + [w_qkv[D + h * HD : D + (h + 1) * HD] for h in hs]
            + [w_qkv[2 * D + h * HD : 2 * D + (h + 1) * HD] for h in hs]
        )
        wqkvT = np.ascontiguousarray(np.concatenate(rows, axis=0).T)  # (1024, 768)
        codeT4 = np.zeros((128, 128), dtype=np.float32)
        for j, h in enumerate(hs):
            hh = j % 2  # position within the hd pair
            ct = fc_code[0, h].T  # (64, 16)
            codeT4[64 * hh : 64 * hh + 64, 32 * j : 32 * j + 16] = ct
        woutT = np.ascontiguousarray(
            np.concatenate([w_out[:, h * HD : (h + 1) * HD].T for h in hs], axis=0)
        ) * np.float32(SCALE)  # (256, 1024), post-softmax scale folded in
        in_maps.append(
            {
                "xT": xTs[b],
                "wqkvT": wqkvT.astype(bf),
                "codeT4": codeT4.astype(bf),
                "woutT": woutT.astype(bf),
                "maskT": mask.astype(bf),
                "iden": iden.astype(bf),
                "onehot": onehot.astype(bf),
            }
        )
    return in_maps


def gather(results):
    out = np.zeros((B, T, D), dtype=np.float32)
    for core in range(N_CORES):
        out[core // HC] += np.asarray(results[core]["partial"], dtype=np.float32)
    return out


_NC_CACHE = {}


def kernel(x, w_qkv, w_out, fc_code):
    from concourse.bass_utils import run_bass_kernel_spmd

    if 1 not in _NC_CACHE:
        _NC_CACHE[1] = build(1)
    nc = _NC_CACHE[1]
    in_maps = make_in_maps(x, w_qkv, w_out, fc_code)
    res = run_bass_kernel_spmd(nc, in_maps, list(range(N_CORES)))
    return gather(res.results)
